# revision 36
# baseline (speedup 1.0000x reference)
"""Trainium2 Bass kernel for nn_AutoCorrelation (AutoCorrelation attention).

Algorithm (per core; data-parallel over batch B=8 across 8 cores):
  1. Q, K viewed as (L=1536, D=H*E=1024), cast to fp16 and transposed on the
     DMA XBAR into (ch-partition, time-free) layout.
  2. Direct circular cross-correlation instead of the FFT round-trip:
     corr[d] = (1/D) sum_ch sum_t k[ch,t] q[ch,(t+d)%L], computed as
     C = K^T Q via PE matmuls that accumulate the 12 time-chunk diagonal
     bands into one (128, 1536) PSUM strip G, where G[p, U] contributes to
     corr[(U - p) % L].
  3. Diagonal reduction via a skewed DRAM view: G written twice into rows of
     width 3073, read back with row stride 3074 -> rows aligned so a plain
     ones-matmul partition reduction yields corr (1, 1536).
  4. AllReduce(sum) of corr (6KB) across the 8 cores -> shared top-7 delay
     indices via DVE max/max_index.
  5. Per-core softmax weights from own corr at the shared indices.
  6. Roll-aggregate out[t] = sum_i w_i * v[(t+s_i) % L] as a block-circulant
     matmul with 12 on-device-built 128x128 weight blocks;
     out_T = sum_U Wblk[(U-T)%12]^T @ V_U, scheduled so every T consumes
     blocks in order g=0,1,2,... (waves of 4 PSUM tiles).
"""

import numpy as np

import concourse.bass as bass
import concourse.mybir as mybir
import concourse.tile as tile
from concourse import bacc
from concourse import bass_utils

B, L, H, E = 8, 1536, 16, 64
D = H * E            # 1024
P = 128
NC = L // P          # 12 time chunks
NH = D // P          # 8 channel chunks
TB = NC + 3          # qT blocks incl. 3 wrap duplicates
TOPK = 7
F32 = mybir.dt.float32
F16 = mybir.dt.float16
MM_DTYPE = mybir.dt.float32r
SKW = 2 * L + 1      # dup-row width for the skew trick

AL = mybir.AluOpType

DEBUG_DUMPS = False
BISECT_NO_V = False      # skip v loads
BISECT_NO_AGG = False    # skip weight blocks + aggregation


def _mm(ap):
    return ap.bitcast(MM_DTYPE) if MM_DTYPE != F32 else ap


def build_program(single_core: bool = False) -> bass.Bass:
    nc = bacc.Bacc(
        "TRN2",
        target_bir_lowering=False,
        debug=False,
        num_devices=1 if single_core else B,
        name="autocorr",
        dynamic_dma_scratch_size=8192,
    )

    q_in = nc.dram_tensor("q", [L, D], F32, kind="ExternalInput")
    k_in = nc.dram_tensor("k", [L, D], F32, kind="ExternalInput")
    v_in = nc.dram_tensor("v", [L, D], F32, kind="ExternalInput")
    out_dram = nc.dram_tensor("out", [L, D], F32, kind="ExternalOutput")
    if DEBUG_DUMPS:
        dbg_corr = nc.dram_tensor("dbg_corr", [1, L], F32, kind="ExternalOutput")
        dbg_g = nc.dram_tensor("dbg_g", [P, L], F32, kind="ExternalOutput")
        dbg_sk = nc.dram_tensor("dbg_sk", [P, L], F32, kind="ExternalOutput")
        dbg_idx = nc.dram_tensor("dbg_idx", [1, 8], F32, kind="ExternalOutput")
        dbg_w = nc.dram_tensor("dbg_w", [1, 8], F32, kind="ExternalOutput")
        dbg_mv = nc.dram_tensor("dbg_mv", [P, NC], F32, kind="ExternalOutput")
        dbg_ones = nc.dram_tensor("dbg_ones", [P, 2], F32, kind="ExternalOutput")
        dbg_c2 = nc.dram_tensor("dbg_c2", [1, L], F32, kind="ExternalOutput")
        dbg_c3 = nc.dram_tensor("dbg_c3", [1, L], F32, kind="ExternalOutput")

    with tile.TileContext(nc) as tc:
        with (
            tc.tile_pool(name="misc", bufs=1) as misc,
            tc.tile_pool(name="dram", bufs=1, space="DRAM") as dram,
            tc.tile_pool(name="outp", bufs=3) as outp,
            tc.tile_pool(name="qkt", bufs=1) as qkt,
            tc.tile_pool(name="vpool", bufs=1) as vpool,
        ):
            qT = qkt.tile([P, TB * NH * P], F16, tag="qT")
            kT = qkt.tile([P, NC * NH * P], F16, tag="kT")
            qT4 = qT[:].rearrange("c (b h t) -> c b h t", b=TB, h=NH)
            kT4 = kT[:].rearrange("c (b h t) -> c b h t", b=NC, h=NH)
            vbig = vpool.tile([P, NC * D], F32, tag="vbig")

            # ---- phase 1: load, cast to fp16, DMA-XBAR transpose ----
            with (
                tc.tile_pool(name="stage", bufs=4) as stage,
                tc.tile_pool(name="cast", bufs=2) as castp,
                tc.tile_pool(name="gpsum", bufs=1, space="PSUM") as gpsum,
            ):
                NG = 3  # chunks per load/cast/transpose group

                def issue_load(src, gi):
                    sf = stage.tile([P, NG * D], F32, tag="sf")
                    nc.sync.dma_start(
                        sf[:].rearrange("p (c m) -> p c m", c=NG),
                        src[gi * NG * P:(gi + 1) * NG * P, :]
                        .rearrange("(c p) m -> p c m", p=P),
                    )
                    return sf

                def cast_transpose(sf, gi, dstT, dup=False):
                    ch = castp.tile([P, NG * D], F16, tag="ch")
                    nc.vector.tensor_copy(ch[:], sf[:])
                    # contiguous (c, 3*8*128) slab; optimizes to
                    # (128, 24, 128) whose transpose semantics give
                    # out[c, 8*ci+hi, t] = in[t, ci*1024 + hi*128 + c]
                    nc.scalar.dma_start(
                        dstT[:, gi * NG:(gi + 1) * NG, :, :],
                        ch[:], transpose=True,
                    )
                    if dup:
                        nc.scalar.dma_start(
                            dstT[:, NC:NC + NG, :, :],
                            ch[:], transpose=True,
                        )

                # software-pipelined: loads run 2 group-pairs ahead of the
                # cast+transpose stage so loads never inherit merged waits
                # on transposes.
                NWAVE = NC // NG
                pend = []
                for gi in range(NWAVE + 2):
                    if gi < NWAVE:
                        sfq = issue_load(q_in, gi)
                        sfk = issue_load(k_in, gi)
                        pend.append((sfq, sfk, gi))
                    if gi >= 2:
                        sfq, sfk, gj = pend[gi - 2]
                        cast_transpose(sfq, gj, qT4, dup=(gj == 0))
                        cast_transpose(sfk, gj, kT4)
                # v loads queue behind the q/k loads on sync (needed by agg
                # only; they fill the DMA idle window after phase 1)
                if not BISECT_NO_V:
                    for half in range(2):
                        hc = NC // 2
                        nc.sync.dma_start(
                            _mm(vbig[:, half * hc * D:(half + 1) * hc * D]
                                .rearrange("p (c m) -> p c m", c=hc)),
                            _mm(v_in[half * hc * P:(half + 1) * hc * P, :]
                                .rearrange("(c p) m -> p c m", p=P)),
                        )

                # ---- phase 2: G strip via K^T Q matmuls ----
                # G[p, 512*gg + u] accumulates over (TI, hi):
                #   sum k[ch, 128*TI + p] * q[ch, (128*(TI+4gg) + u) % L]
                gps = gpsum.tile([P, L], F32, tag="gps")

                def dep(TI, gg):
                    s = (TI + 4 * gg) % NC
                    dq = max(2 * ((b if b < NC else b - NC) // NG)
                             for b in range(s, s + 4))
                    return max(dq, 2 * (TI // NG) + 1)

                steps = sorted(
                    [(TI, gg) for TI in range(NC) for gg in range(3)],
                    key=lambda x: (dep(*x), x[0], x[1]),
                )
                n_seen = [0] * 3
                n_total = [NC * NH] * 3
                for TI, gg in steps:
                    s = (TI + 4 * gg) % NC
                    for hi in range(NH):
                        st = (n_seen[gg] == 0)
                        n_seen[gg] += 1
                        sp = (n_seen[gg] == n_total[gg])
                        nc.tensor.matmul(
                            gps[:, gg * 512:(gg + 1) * 512],
                            kT4[:, TI, hi, :],
                            qT4[:, s:s + 4, hi, :],
                            start=st, stop=sp,
                        )

                # ---- phase 3: skew reduction -> corr (1, L) ----
                gsb = misc.tile([P, L], F32, tag="gsb")
                nc.scalar.copy(gsb[:, 0:768], gps[:, 0:768])
                nc.vector.tensor_copy(gsb[:, 768:1536], gps[:, 768:1536])

            fl = dram.tile([P * (SKW + 1)], F32)
            wview = fl[0:P * SKW].rearrange("(p c) -> p c", c=SKW)
            rview = fl[:].rearrange("(p c) -> p c", c=SKW + 1)
            nc.sync.dma_start(
                wview[:, 0:2 * L].rearrange("p (r c) -> p r c", r=2),
                gsb[:].unsqueeze(1).to_broadcast((P, 2, L)),
            )
            sk = misc.tile([P, L], F32, tag="sk")
            nc.sync.dma_start(_mm(sk[:]), _mm(rview[:, 0:L]))

            ones1 = misc.tile([P, 1], F32, tag="ones1")
            nc.vector.memset(ones1[:], 1.0)
            onesD = misc.tile([P, 1], F32, tag="onesD")
            nc.vector.tensor_scalar(
                out=_mm(onesD[:]), in0=ones1[:], scalar1=1.0 / D,
                scalar2=None, op0=AL.mult,
            )
            csb = misc.tile([1, L], F32, tag="csb")
            with tc.tile_pool(name="cpsum", bufs=1, space="PSUM") as cpsum:
                cps = cpsum.tile([1, L], F32, tag="cps")
                for j in range(3):
                    nc.tensor.matmul(
                        cps[0:1, j * 512:(j + 1) * 512],
                        _mm(onesD[:]), _mm(sk[:, j * 512:(j + 1) * 512]),
                        start=True, stop=True,
                    )
                nc.scalar.copy(csb[0:1, :], cps[0:1, :])

            # ---- allreduce corr across cores; own corr -> (128, 12) ----
            cc_in = dram.tile([L], F32)
            cc_out = dram.tile([L], F32)
            nc.sync.dma_start(cc_in[:].unsqueeze(0), csb[0:1, :])
            if single_core:
                nc.sync.dma_start(cc_out[:], cc_in[:])
            else:
                nc.gpsimd.collective_compute(
                    "AllReduce",
                    AL.add,
                    replica_groups=[list(range(B))],
                    ins=[cc_in[:].opt()],
                    outs=[cc_out[:].opt()],
                )
            bm = misc.tile([1, L], F32, tag="bm")
            nc.sync.dma_start(bm[0:1, :], cc_out[:].unsqueeze(0))
            # own corr re-layout (128, 12) via PE transposes (no strided DMA)
            onesrow = misc.tile([1, P], F32, tag="onesrow")
            nc.vector.memset(onesrow[0:1, :], 1.0)
            id11 = misc.tile([1, 1], F32, tag="id11")
            nc.vector.memset(id11[0:1, 0:1], 1.0)
            mv2d = misc.tile([P, NC], F32, tag="mv2d")
            with tc.tile_pool(name="tpsum", bufs=1, space="PSUM") as tpsum:
                mvps = tpsum.tile([P, NC], F32, tag="mvps")
                for c in range(NC):
                    nc.tensor.matmul(
                        mvps[:, c:c + 1], csb[0:1, c * P:(c + 1) * P],
                        id11[0:1, 0:1], is_transpose=True,
                        start=True, stop=True,
                    )
                nc.scalar.copy(mv2d[:], mvps[:])

            # ---- top-7 indices from batch-summed corr ----
            top8 = misc.tile([1, 8], F32, tag="top8")
            idx8 = misc.tile([1, 8], mybir.dt.uint32, tag="idx8")
            idxf = misc.tile([1, 8], F32, tag="idxf")
            nc.vector.max(top8[:], bm[0:1, :])
            nc.vector.max_index(idx8[:], top8[:], bm[0:1, :])
            nc.vector.tensor_copy(idxf[:], idx8[:])
            if DEBUG_DUMPS:
                # repeat the reduction late, and with a fresh ones tile
                onesF = misc.tile([P, 1], F32, tag="onesF")
                nc.vector.memset(onesF[:], 1.0)
                onesF2 = misc.tile([P, 1], F32, tag="onesF2")
                nc.vector.tensor_scalar(
                    out=_mm(onesF2[:]), in0=onesF[:], scalar1=1.0 / D,
                    scalar2=None, op0=AL.mult,
                )
                csb2 = misc.tile([1, L], F32, tag="csb2")
                csb3 = misc.tile([1, L], F32, tag="csb3")
                with tc.tile_pool(name="c2psum", bufs=1, space="PSUM") as c2p:
                    cps2 = c2p.tile([1, L], F32, tag="cps2")
                    for j in range(3):
                        nc.tensor.matmul(
                            cps2[0:1, j * 512:(j + 1) * 512],
                            _mm(onesD[:]), _mm(sk[:, j * 512:(j + 1) * 512]),
                            start=True, stop=True,
                        )
                    nc.scalar.copy(csb2[0:1, :], cps2[0:1, :])
                    cps3 = c2p.tile([1, L], F32, tag="cps3")
                    for j in range(3):
                        nc.tensor.matmul(
                            cps3[0:1, j * 512:(j + 1) * 512],
                            _mm(onesF2[:]), _mm(sk[:, j * 512:(j + 1) * 512]),
                            start=True, stop=True,
                        )
                    nc.scalar.copy(csb3[0:1, :], cps3[0:1, :])
                nc.sync.dma_start(dbg_c2[0:1, :], csb2[0:1, :])
                nc.sync.dma_start(dbg_c3[0:1, :], csb3[0:1, :])
                nc.sync.dma_start(dbg_g[:, :], gsb[:])
                nc.sync.dma_start(dbg_ones[:, 0:1], ones1[:])
                nc.sync.dma_start(dbg_ones[:, 1:2], onesD[:])
                nc.sync.dma_start(dbg_sk[:, :], sk[:])
                nc.sync.dma_start(dbg_corr[0:1, :], csb[0:1, :])
                nc.sync.dma_start(dbg_idx[0:1, :], idxf[0:1, :])
                nc.sync.dma_start(dbg_mv[:, :], mv2d[:])

            # ---- per-core weights: softmax(own corr at idx[0..6]) ----
            # broadcast idxf to all partitions via PE ones-outer-product
            irep = misc.tile([P, 8], F32, tag="irep")
            with tc.tile_pool(name="bpsum", bufs=1, space="PSUM") as bpsum:
                irps = bpsum.tile([P, 8], F32, tag="irps")
                nc.tensor.matmul(
                    irps[:, :], onesrow[0:1, :], idxf[0:1, :],
                    start=True, stop=True,
                )
                nc.scalar.copy(irep[:], irps[:])
            iota2dg = misc.tile([P, NC], F32, tag="iota2dg")
            nc.gpsimd.iota(
                iota2dg[:], pattern=[[P, NC]], base=0, channel_multiplier=1,
                allow_small_or_imprecise_dtypes=True,
            )  # iota2d[p, c] = p + 128*c
            iota2d = misc.tile([P, NC], F32, tag="iota2d")
            nc.vector.tensor_copy(iota2d[:], iota2dg[:])
            irepv = misc.tile([P, 8], F32, tag="irepv")
            nc.vector.tensor_copy(irepv[:], irep[:])
            oh2d = misc.tile([P, NC], F32, tag="oh2d")
            rgat = misc.tile([P, 8], F32, tag="rgat")
            for i in range(TOPK):
                nc.vector.tensor_scalar(
                    out=oh2d[:], in0=iota2d[:], scalar1=irepv[:, i:i + 1],
                    scalar2=None, op0=AL.is_equal,
                )
                nc.vector.tensor_tensor(oh2d[:], oh2d[:], mv2d[:], AL.mult)
                nc.vector.tensor_reduce(
                    out=rgat[:, i:i + 1], in_=oh2d[:],
                    axis=mybir.AxisListType.X, op=AL.add,
                )
            wraw = misc.tile([1, 8], F32, tag="wraw")
            with tc.tile_pool(name="midpsum", bufs=1, space="PSUM") as midpsum:
                wps = midpsum.tile([1, 8], F32, tag="wps")
                nc.tensor.matmul(
                    wps[0:1, 0:TOPK], ones1[:], rgat[:, 0:TOPK],
                    start=True, stop=True,
                )
                nc.scalar.copy(wraw[0:1, 0:TOPK], wps[0:1, 0:TOPK])
            negmax = misc.tile([1, 1], F32, tag="negmax")
            nc.vector.tensor_reduce(
                out=negmax[0:1, 0:1], in_=wraw[0:1, 0:TOPK],
                axis=mybir.AxisListType.X, op=AL.max, negate=True,
            )
            negmax2 = misc.tile([1, 1], F32, tag="negmax2")
            nc.scalar.copy(negmax2[0:1, 0:1], negmax[0:1, 0:1])
            ew = misc.tile([1, 8], F32, tag="ew")
            sumw = misc.tile([1, 1], F32, tag="sumw")
            nc.scalar.activation(
                out=ew[0:1, 0:TOPK], in_=wraw[0:1, 0:TOPK],
                func=mybir.ActivationFunctionType.Exp,
                bias=negmax2[0:1, 0:1], scale=1.0,
                accum_out=sumw[0:1, 0:1],
            )
            rsum = misc.tile([1, 1], F32, tag="rsum")
            nc.vector.reciprocal(rsum[0:1, 0:1], sumw[0:1, 0:1])
            wvec = misc.tile([1, 8], F32, tag="wvec")
            nc.vector.tensor_scalar(
                out=wvec[0:1, 0:TOPK], in0=ew[0:1, 0:TOPK],
                scalar1=rsum[0:1, 0:1], scalar2=None, op0=AL.mult,
            )
            if DEBUG_DUMPS:
                nc.sync.dma_start(dbg_w[0:1, :], wvec[0:1, :])

            # ---- v_tab: wrapped shift reps per (g, i): (1, 12*7) ----
            giofg = misc.tile([1, NC * TOPK], F32, tag="giofg")
            nc.gpsimd.iota(
                giofg[0:1, :].rearrange("o (g i) -> o g i", g=NC),
                pattern=[[-P, NC], [0, TOPK]], base=0, channel_multiplier=0,
                allow_small_or_imprecise_dtypes=True,
            )  # giof[0, g*7+i] = -128*g
            giof = misc.tile([1, NC * TOPK], F32, tag="giof")
            nc.vector.tensor_copy(giof[0:1, :], giofg[0:1, :])
            vt = misc.tile([1, NC * TOPK], F32, tag="vt")
            for g in range(NC):
                nc.vector.tensor_copy(
                    vt[0:1, g * TOPK:(g + 1) * TOPK], idxf[0:1, 0:TOPK]
                )
            nc.vector.tensor_tensor(vt[:], vt[:], giof[:], AL.add)
            cwrap = misc.tile([1, NC * TOPK], F32, tag="cwrap")
            nc.vector.tensor_scalar(
                out=cwrap[:], in0=vt[:], scalar1=-768.0, scalar2=1536.0,
                op0=AL.is_lt, op1=AL.mult,
            )
            nc.vector.tensor_tensor(vt[:], vt[:], cwrap[:], AL.add)
            nc.vector.tensor_scalar(
                out=cwrap[:], in0=vt[:], scalar1=768.0, scalar2=1536.0,
                op0=AL.is_ge, op1=AL.mult,
            )
            nc.vector.tensor_tensor(vt[:], vt[:], cwrap[:], AL.subtract)

            # replicate v_tab and weights to all partitions via PE broadcast
            vrep = misc.tile([P, NC * TOPK], F32, tag="vrep")
            wrep = misc.tile([P, TOPK], F32, tag="wrep")
            with tc.tile_pool(name="b2psum", bufs=1, space="PSUM") as b2psum:
                vrps = b2psum.tile([P, NC * TOPK], F32, tag="vrps")
                nc.tensor.matmul(
                    vrps[:, :], onesrow[0:1, :], vt[0:1, :],
                    start=True, stop=True,
                )
                nc.scalar.copy(vrep[:], vrps[:])
                wrps = b2psum.tile([P, TOPK], F32, tag="wrps")
                nc.tensor.matmul(
                    wrps[:, :], onesrow[0:1, :], wvec[0:1, 0:TOPK],
                    start=True, stop=True,
                )
                nc.vector.tensor_copy(wrep[:, 0:TOPK], wrps[:, :])

            if BISECT_NO_AGG:
                for T in range(NC):
                    nc.sync.dma_start(out_dram[T * P:(T + 1) * P, :],
                                      sk[:, 0:D])
                return_early = True
            else:
                return_early = False
            # ---- build the 12 circulant weight blocks (g ascending) ----
            # per-engine staging of the pointer operands (vrep/wrep/af):
            # a same-queue copy makes pointer-operand races impossible.
            afp = misc.tile([P, P], F32, tag="afp")
            nc.gpsimd.iota(
                afp[:], pattern=[[-1, P]], base=0, channel_multiplier=1,
                allow_small_or_imprecise_dtypes=True,
            )  # af[p, j] = p - j
            afv = misc.tile([P, P], F32, tag="afv")
            nc.vector.tensor_copy(afv[:], afp[:])
            vrepp = misc.tile([P, NC * TOPK], F32, tag="vrepp")
            nc.gpsimd.tensor_copy(vrepp[:], vrep[:])
            wrepp = misc.tile([P, TOPK], F32, tag="wrepp")
            nc.gpsimd.tensor_copy(wrepp[:], wrep[:])
            vrepv = misc.tile([P, NC * TOPK], F32, tag="vrepv")
            nc.vector.tensor_copy(vrepv[:], vrep[:])
            wrepv = misc.tile([P, TOPK], F32, tag="wrepv")
            nc.vector.tensor_copy(wrepv[:], wrep[:])
            tmpw = misc.tile([P, P], F32, tag="tmpw")
            tmpw2 = misc.tile([P, P], F32, tag="tmpw2")
            wblk = [
                misc.tile([P, P], F32, tag=f"wblk{g}", name=f"wblk{g}")
                for g in range(NC)
            ]
            for g in range(NC if not return_early else 0):
                onv = (g % 2 == 0)
                eng = nc.vector if onv else nc.gpsimd
                tw = tmpw if onv else tmpw2
                afx = afv if onv else afp
                vrx = vrepv if onv else vrepp
                wrx = wrepv if onv else wrepp
                for i in range(TOPK):
                    dst = _mm(wblk[g][:]) if i == 0 else tw[:]
                    eng.tensor_scalar(
                        out=dst, in0=afx[:],
                        scalar1=vrx[:, g * TOPK + i:g * TOPK + i + 1],
                        scalar2=wrx[:, i:i + 1],
                        op0=AL.is_equal, op1=AL.mult,
                    )
                    if i > 0:
                        eng.tensor_tensor(
                            _mm(wblk[g][:]), wblk[g][:], tw[:], AL.add
                        )

            # ---- aggregation: out_T = sum_U Wblk[(U-T)%12]^T @ V_U ----
            # waves of 4 PSUM tiles; within a wave, blocks consumed in order
            # g = 0, 1, 2, ... so the build race stays ahead.
            with tc.tile_pool(name="aggpsum", bufs=4, space="PSUM") as aggpsum:
                for wave in range(3 if not return_early else 0):
                    pos = [aggpsum.tile([P, D], F32, tag="agg",
                                        name=f"agg{wave}_{ti}")
                           for ti in range(4)]
                    for g in range(NC):
                        for ti in range(4):
                            T = wave * 4 + ti
                            U = (T + g) % NC
                            st, sp = (g == 0), (g == NC - 1)
                            for nh in range(2):
                                sl = slice(nh * 512, (nh + 1) * 512)
                                nc.tensor.matmul(
                                    pos[ti][:, sl], _mm(wblk[g][:]),
                                    _mm(vbig[:, U * D + nh * 512:
                                             U * D + (nh + 1) * 512]),
                                    start=st, stop=sp,
                                )
                    for ti in range(4):
                        T = wave * 4 + ti
                        ot = outp.tile([P, D], F32, tag="ot")
                        nc.scalar.copy(ot[:], pos[ti][:])
                        nc.sync.dma_start(
                            out_dram[T * P:(T + 1) * P, :], ot[:]
                        )

    nc.compile()
    return nc


_prog_cache = None


def _get_program():
    global _prog_cache
    if _prog_cache is None:
        _prog_cache = build_program()
    return _prog_cache


def kernel(queries, keys, values, attn_mask=0):
    nc = _get_program()
    q = np.ascontiguousarray(np.asarray(queries, dtype=np.float32).reshape(B, L, D))
    k = np.ascontiguousarray(np.asarray(keys, dtype=np.float32).reshape(B, L, D))
    v = np.ascontiguousarray(np.asarray(values, dtype=np.float32).reshape(B, L, D))
    in_maps = [{"q": q[c], "k": k[c], "v": v[c]} for c in range(B)]
    res = bass_utils.run_bass_kernel_spmd(nc, in_maps, core_ids=list(range(B)))
    out = np.stack([res.results[c]["out"] for c in range(B)])
    return out.reshape(B, L, H, E)


if __name__ == "__main__":
    prog = build_program(single_core=True)
    print("program built ok")
